# revision 17
# baseline (speedup 1.0000x reference)
"""Kent-distribution pairwise KLD loss kernel for Trainium2 (8 NeuronCores).

The [N, M] pairwise KLD matrix factors exactly as a rank-11 product
U @ V^T:

  KLD[n, m] = A[n]                                  (pred-row constant)
            + c_b[m]                                (target-row constant)
            - Ex_a[n] . (kappa_b[m] * gamma_b1[m])  (rank 3)
            + <ExxT_a[n], beta_b[m]*(g_b3 g_b3^T - g_b2 g_b2^T)>  (rank 6, sym)

so each core computes 11 features per pred row (its N-shard) and per
target row (replicated), then skinny matmuls [11,128]^T @ [11,512].
N is sharded across the 8 cores (data parallel over predictions).

v2 performance notes (vs the 41us fp32 baseline):
 - features are rounded to fp16 just before the transpose stage, so
   transposes and the main matmuls run at 1 cycle/row instead of fp32's
   LOW/HIGH 4 cycles/row double-pass (error ~5e-4 vs the 2e-2 gate).
 - the PE is warmed with 3 junk matmuls on the identity during the DVE
   feature phase, so transposes + matmuls run at 2.4 GHz instead of the
   HAM-throttled 1.2 GHz.
 - the feature chain is restructured: batched strided-AP muls compute
   all gamma entries in ~9 vector ops (g2 = cp*P + sp*V', g3 = -sp*P +
   cp*V' with P = (-sa, ca ce, ca se), V' = (-se, ce)); |gamma_1|^2 == 1
   and the ln(2pi) cancellation remove further ops; the kappa/lambda
   chain runs on GpSimd in parallel with the gamma chain on Vector.
 - output is written as fp16 (1MB instead of 2MB of HBM writes; the
   host upcasts) - error budget ~1e-3 vs the 2e-2 gate.

Numerics (validated against the jax reference):
 - exp(c_k - c), exp(c_kk - c) are evaluated as exact algebraic ratios:
   l1 = (k^2-k-4b^2)/D * e^-EPS, l2 = 0.5(2k^3-2k^2-2sk-s)/D^2 with
   D = k^2-4b^2, s = 4b^2 (no Exp table; DVE reciprocal instead).
 - exp(c_beta - c) carries e^-kappa <= 4.5e-5 (kappa >= 10) and is
   dropped; with lambda2 == lambda3, ExxT = l2*I + (l1-l2)*g1 g1^T and
   the beta_a*(qa2-qa3) term vanishes.
 - Sin's HW domain is [-pi, pi]: cos(x) = sin(pi/2 - |x|), |x| on DVE.
"""

import sys

import numpy as np

sys.path.insert(0, "/opt/trn_rl_repo")

import concourse.bass as bass  # noqa: E402,F401
import concourse.mybir as mybir  # noqa: E402
import concourse.tile as tile  # noqa: E402
from concourse import bacc  # noqa: E402
from concourse.masks import make_identity  # noqa: E402

F32 = mybir.dt.float32
F32R = mybir.dt.float32r
F16 = mybir.dt.float16
AF = mybir.ActivationFunctionType
ALU = mybir.AluOpType

N = 2048
M = 2048
NCORES = 8
NS = N // NCORES  # 256 pred rows per core
K = 11  # feature rank
GP = NS // 128  # pred row-groups (2)
GT = M // 128  # target row-groups (16)
G = GP + GT  # 18

PI = float(np.pi)
EPS = 1e-6
EM = float(np.exp(-1e-6))  # e^-EPS factor from the reference's den EPS


def _body(tc, pred, targ, out):
    nc = tc.nc
    with (
        tc.tile_pool(name="main", bufs=1) as pool,
        tc.tile_pool(name="junk_psum", bufs=1, space="PSUM") as jpp,
        tc.tile_pool(name="tp_psum", bufs=2, space="PSUM") as tpp,
        tc.tile_pool(name="ut_psum", bufs=1, space="PSUM") as upp,
        tc.tile_pool(name="out_psum", bufs=3, space="PSUM") as opp,
    ):
        def t(shape, tag, dtype=F32):
            return pool.tile([128, *shape], dtype, name=tag, tag=tag)

        def vmul(o, a, b):
            nc.vector.tensor_mul(o, a, b)

        def gmul(o, a, b):
            nc.gpsimd.tensor_mul(o, a, b)

        def gadd(o, a, b):
            nc.gpsimd.tensor_add(o, a, b)

        def gsub(o, a, b):
            nc.gpsimd.tensor_sub(o, a, b)

        def vstt(o, in0, scalar, in1, op0, op1):
            nc.vector.scalar_tensor_tensor(o, in0, scalar, in1, op0, op1)

        # ---- identity (for PE transposes) + PE warm-up.  The HAM
        # clock-gate opens (1.2 -> 2.4 GHz) only after one complete
        # free-running 4096-cycle (3.4us) window of sustained PE
        # activity; 22 junk matmuls (~7us cold) guarantee a full window
        # regardless of phase, so the real transposes/matmuls run warm.
        ident = t([128], "ident", F16)
        make_identity(nc, ident)
        junk = jpp.tile([128, 384], F32, name="junk", tag="junk")
        ident_wide = ident[:].unsqueeze(1).broadcast_to([128, 3, 128])
        for _ in range(16):
            nc.tensor.matmul(junk[:], ident[:], ident_wide, start=True, stop=True)
        # finer-grained tail junks: small over-shoot if the feature phase
        # finishes first, while still bridging the HAM MID window
        for _ in range(24):
            nc.tensor.matmul(
                junk[:, 0:128], ident[:], ident[:], start=True, stop=True
            )

        # ---- load params: pred partition p holds rows 2p,2p+1; targ
        # partition p holds rows 16p..16p+15 (contiguous per-partition DMA).
        params = t([G * 5], "params")
        # j-major input layout: row r lands at (partition r % 128, slot
        # r // 128), so each transposed group j is the contiguous output
        # block [128 j, 128 j + 128) -- VT copies and the main matmul
        # moving operand stay fully contiguous.  The DRAM side becomes a
        # 20 B-granular gather (2048 descriptors ~ 0.9 us across the 16
        # SDMA engines), paid once during the preamble shadow.
        # The two DMAs issue from different engines (Act + SP HWDGE
        # rings) so their ~2us completion latencies overlap.
        nc.scalar.dma_start(
            out=params[:, 0 : GP * 5].rearrange("p (j c) -> p j c", c=5),
            in_=pred.rearrange("(j p) c -> p j c", p=128),
        )
        nc.sync.dma_start(
            out=params[:, GP * 5 : G * 5].rearrange("p (j c) -> p j c", c=5),
            in_=targ.rearrange("(j p) c -> p j c", p=128),
        )

        P5 = params.rearrange("p (g c) -> p c g", c=5)  # [128, 5, 18]
        kap = P5[:, 3, :]  # [128, 18] stride-5 slabs
        bet = P5[:, 4, :]
        kap_p, kap_t = kap[:, 0:GP], kap[:, GP:G]
        bet_t = bet[:, GP:G]

        half_pi = pool.tile([128, 1], F32, name="half_pi", tag="half_pi")
        nc.vector.memset(half_pi, PI / 2)
        eps_c = pool.tile([128, 1], F32, name="eps_c", tag="eps_c")
        nc.vector.memset(eps_c, EPS)
        # dummy Sin on a constant: hoists the trig ACT_TABLE_LOAD off the
        # input-DMA critical path (runs while the DMA is in flight)
        sin_dummy = pool.tile([128, 1], F32, name="sin_dummy", tag="sin_dummy")
        nc.scalar.activation(sin_dummy[:], half_pi[:], AF.Sin)
        # (no dummy Ln: the trig and natural-log act-table sets evict each
        # other, so any Sin->Ln interleave forces 1.28us reloads; the one
        # real Ln below is the single switch this kernel pays for)

        # ---- trig: TRIG rows (se, sa, sp, ce, ca, cp, -se, pad, -sp).
        # The row spacing is chosen so later ops can read pairs like
        # (cp,-sp), (sp,cp), (ce,se), (-se,ce) with stride-3 APs.
        angles = P5[:, 0:3, :]  # [128, 3, 18] (eta, alpha, psi)
        TRIG = t([9, G], "TRIG")
        absv = t([3, G], "absv")
        vstt(absv[:], angles, -1.0, angles, ALU.mult, ALU.max)
        nc.scalar.activation(TRIG[:, 0:3, :], angles, AF.Sin)
        nc.scalar.activation(TRIG[:, 3:6, :], absv[:], AF.Sin, bias=half_pi, scale=-1.0)
        nc.scalar.activation(TRIG[:, 6::2, :], P5[:, 0:3:2, :], AF.Sin, scale=-1.0)
        se = TRIG[:, 0, :]
        sa = TRIG[:, 1, :]
        ce = TRIG[:, 3, :]
        ca = TRIG[:, 4, :]
        ce_se = TRIG[:, 3::-3, :]  # rows (3,0) = (ce, se)
        cp_nsp = TRIG[:, 5::3, :]  # rows (5,8) = (cp, -sp)
        sp_cp = TRIG[:, 2:6:3, :]  # rows (2,5) = (sp, cp)
        nse_ce = TRIG[:, 6:2:-3, :]  # rows (6,3) = (-se, ce)

        # ---- kappa/beta chain on gpsimd (parallel with trig/gamma on
        # vector): c = ln2pi + k - 0.5 ln((k-2b)(k+2b)+EPS); the ln2pi
        # cancels between c_a and c_b and is dropped from both sides.
        b2 = t([G], "b2")
        gadd(b2, bet, bet)  # 2*beta
        b2_t = b2[:, GP:G]
        x2f = t([G], "x2f")
        gmul(x2f, kap, kap)
        b2sq = t([G], "b2sq")
        gmul(b2sq, b2, b2)
        # LNIN = (k-2b)(k+2b) = k^2 - 4b^2; its pred slice IS the lambda
        # denominator D, and x2f/b2sq pred slices are x2/s reused below
        LNIN = t([G], "LNIN")
        gsub(LNIN[:], x2f, b2sq)
        LNOUT = t([G], "LNOUT")
        nc.scalar.activation(LNOUT[:], LNIN[:], AF.Ln, bias=eps_c)
        lnprod_p = LNOUT[:, 0:GP]
        lnprod_t = LNOUT[:, GP:G]

        # pred-only lambda chain prefix (gpsimd).  GpSimd has no
        # scalar_tensor_tensor / tensor_scalar ISA support, so the chain
        # is formulated sign-flipped in pure tensor_tensor ops:
        #   negn = -(k^2 - k - s),  l1n = -l1 = negn/D,
        #   Qhn = k*negn + s/2 = -Q/2,  l2n = -l2 = Qhn/D^2,
        #   dEn = l1n - l2n = -(l1 - l2)
        # (the e^-EPS factor on l1 is dropped: relative 1e-6, far below
        # the fp16 feature rounding)
        b2_p = b2[:, 0:GP]
        bet_p = bet[:, 0:GP]
        x2 = x2f[:, 0:GP]  # k^2
        s4 = b2sq[:, 0:GP]  # s = 4 b^2
        D = LNIN[:, 0:GP]  # k^2 - 4 b^2
        sh = t([GP], "sh")
        gmul(sh, b2_p, bet_p)  # s/2 = 2 b^2
        tneg = t([GP], "tneg")
        gsub(tneg, x2, kap_p)
        negn = t([GP], "negn")
        gsub(negn, s4, tneg)  # -(k^2 - k - s)

        # ---- gamma vectors (vector engine), batched:
        #   g1 = (ca, sa ce, sa se)
        #   g2 = cp*P + sp*V',  g3 = -sp*P + cp*V'
        # with P = (-sa, ca ce, ca se), V' = (-se, ce) (comps 1:3).
        gam = t([3, 3, G], "gam")  # [row, comp, group]
        vmul(gam[:, 0, 1:3, :], sa.unsqueeze(1).broadcast_to([128, 2, G]), ce_se)
        P = t([3, G], "P")
        vmul(P[:, 1:3, :], ca.unsqueeze(1).broadcast_to([128, 2, G]), ce_se)
        nc.vector.tensor_scalar_mul(P[:, 0, :], sa, -1.0)
        nc.vector.tensor_copy(gam[:, 0, 0, :], ca)
        # r = 1/D early (vector owns reciprocal; gpsimd lambda chain waits)
        r = t([GP], "r")
        nc.vector.reciprocal(r, D)
        r2 = t([GP], "r2")
        vmul(r2, r, r)
        # X = (cp*P ; -sp*P) written straight into gam rows 2-3; Y is
        # then added in place on the 1:3 components
        vmul(
            gam[:, 1:3, :, :],
            cp_nsp.unsqueeze(2).broadcast_to([128, 2, 3, G]),
            P[:].unsqueeze(1).broadcast_to([128, 2, 3, G]),
        )
        Y = t([2, 2, G], "Y")  # (sp*V' ; cp*V')
        vmul(
            Y[:],
            sp_cp.unsqueeze(2).broadcast_to([128, 2, 2, G]),
            nse_ce.unsqueeze(1).broadcast_to([128, 2, 2, G]),
        )
        nc.vector.tensor_add(gam[:, 1:3, 1:3, :], gam[:, 1:3, 1:3, :], Y[:])

        # ---- pair products p_c_e for e in [00,11,22,01,02,12]
        prod = t([3, 6, G], "prod")
        vmul(prod[:, :, 0:3, :], gam[:], gam[:])
        vmul(
            prod[:, :, 3:5, :],
            gam[:, :, 0:1, :].broadcast_to([128, 3, 2, G]),
            gam[:, :, 1:3, :],
        )
        vmul(prod[:, :, 5, :], gam[:, :, 1, :], gam[:, :, 2, :])
        # dVn = -(g3 x g3 - g2 x g2); pairs with the negated ExxT rows in UF
        dVn = t([6, GT], "dVn")
        nc.vector.tensor_sub(dVn, prod[:, 1, :, GP:G], prod[:, 2, :, GP:G])

        # ---- gpsimd lambda chain (sign-flipped, see above)
        l1n = t([GP], "l1n")
        gmul(l1n, negn, r[:])  # -l1
        kneg = t([GP], "kneg")
        gmul(kneg, kap_p, negn)
        Qhn = t([GP], "Qhn")
        gadd(Qhn, kneg, sh)  # -0.5 Q
        l2n = t([GP], "l2n")
        gmul(l2n, Qhn, r2[:])  # -l2
        dEn = t([GP], "dEn")
        gsub(dEn, l1n, l2n)  # -(l1 - l2)

        l1nb = l1n[:].unsqueeze(1)
        l2nb = l2n[:].unsqueeze(1)
        dEnb = dEn[:].unsqueeze(1)

        # ---- target features, split into TWO tiles (Tile tracks deps at
        # tile granularity): vector assembles VFa (groups 0-7), gpsimd
        # VFb (groups 8-15), so transposes of the front half need not
        # wait for the back half.  VFb's muls are emitted after the UF
        # section below so UT unblocks first.
        H = GT // 2
        VFa = t([K, H], "VFa", F16)
        VFb = t([K, H], "VFb", F16)
        nc.gpsimd.memset(VFb[:, 0, :], 1.0)
        nc.vector.memset(VFa[:, 0, :], 1.0)
        # c_b (ln2pi dropped; cancels against c_a)
        vstt(VFa[:, 1, :], lnprod_t[:, 0:H], -0.5, kap_t[:, 0:H], ALU.mult, ALU.add)
        vstt(VFb[:, 1, :], lnprod_t[:, H:GT], -0.5, kap_t[:, H:GT], ALU.mult, ALU.add)
        kb3 = kap_t.unsqueeze(1).broadcast_to([128, 3, GT])
        bb3 = bet_t.unsqueeze(1).broadcast_to([128, 3, GT])
        b2b3 = b2_t.unsqueeze(1).broadcast_to([128, 3, GT])
        gam1t = gam[:, 0, :, GP:G]
        vmul(VFa[:, 2:5, :], gam1t[:, :, 0:H], kb3[:, :, 0:H])
        vmul(VFa[:, 5:8, :], dVn[:, 0:3, 0:H], bb3[:, :, 0:H])
        vmul(VFa[:, 8:11, :], dVn[:, 3:6, 0:H], b2b3[:, :, 0:H])

        # ---- pred features UF [128, 11, 2] (gpsimd; a1 on vector, the
        # only op here needing an immediate-scalar ALU)
        UF = t([K, GP], "UF", F16)
        nc.gpsimd.memset(UF[:, 1, :], 1.0)
        # Ex_a with the pairing minus sign folded in: -l1 * g1 = l1n * g1
        vmul(UF[:, 2:5, :], gam[:, 0, :, 0:GP], l1nb.broadcast_to([128, 3, GP]))
        # -ExxT = l2n I + dEn g1 g1^T  (sign matched by dVn on the V side)
        edtn = t([3, GP], "edtn")
        gmul(edtn, prod[:, 0, 0:3, 0:GP], dEnb.broadcast_to([128, 3, GP]))
        gadd(UF[:, 5:8, :], edtn, l2nb.broadcast_to([128, 3, GP]))
        vmul(UF[:, 8:11, :], prod[:, 0, 3:6, 0:GP], dEnb.broadcast_to([128, 3, GP]))
        # A = (0.5 lnprod - k) + k l1  (|gamma_1|^2 == 1 exactly)
        kadotn = t([GP], "kadotn")
        gmul(kadotn, kap_p, l1n)  # -k l1
        a1 = t([GP], "a1")
        vstt(a1, lnprod_p, 0.5, kap_p, ALU.mult, ALU.subtract)
        gsub(UF[:, 0, :], a1, kadotn)

        # back-half target features (gpsimd), after UF so UT unblocks first
        gmul(VFb[:, 2:5, :], gam1t[:, :, H:GT], kb3[:, :, H:GT])
        gmul(VFb[:, 5:8, :], dVn[:, 0:3, H:GT], bb3[:, :, H:GT])
        gmul(VFb[:, 8:11, :], dVn[:, 3:6, H:GT], b2b3[:, :, H:GT])

        # ---- transpose targets to group-major VT [11, 2048] (col = 128j + p)
        # (PSUM -> SBUF copies must stay on scalar/vector: GpSimd has no
        # PSUM access.)  The UT transposes slot between the two VF halves.
        VT = pool.tile([K, M], F16, name="VT", tag="VT")
        UT = pool.tile([K, NS], F16, name="UT", tag="UT")
        cp_engines = [nc.scalar.copy, nc.vector.tensor_copy]

        # with the j-major input layout, group j's transpose IS the
        # contiguous output block m in [128 j, 128 j + 128), and main-
        # matmul chunk c needs only VT quarter c -- so each quarter runs
        # its own transpose -> copy -> matmul -> out-copy -> DMA pipeline
        # instead of waiting for all of VT.

        # preds first: UT col = pred row (j-major load), plain copy
        utp = upp.tile([K, GP * 128], F16, name="utp", tag="utp")
        for j in range(GP):
            nc.tensor.transpose(utp[:, j * 128 : (j + 1) * 128], UF[:, :, j], ident[:])
        nc.vector.tensor_copy(UT[:], utp[:])

        UTr = UT[:]
        outv = out.rearrange("(t p) m -> p t m", p=128)  # row = 128 t + p
        for q in range(4):
            vtp = tpp.tile([K, 512], F16, name="vtp", tag="vtp")
            VFh = VFa if q < 2 else VFb
            for jj in range(4):
                j = (q % 2) * 4 + jj
                nc.tensor.transpose(
                    vtp[:, jj * 128 : (jj + 1) * 128], VFh[:, :, j], ident[:]
                )
            cp_engines[q % 2](VT[:, q * 512 : (q + 1) * 512], vtp[:])
            out_sb = pool.tile([128, GP, 512], F16, name="out_sb", tag="out_sb", bufs=4)
            for ti in range(GP):
                ops = opp.tile([128, 512], F32, name="ops", tag="ops", bufs=4)
                # 4x128-col matmuls pipeline at ~56 ns pace on the PE
                # queue; a single 512-col matmul runs ~374 ns
                for u in range(4):
                    nc.tensor.matmul(
                        ops[:, 128 * u : 128 * (u + 1)],
                        UTr[:, 128 * ti : 128 * (ti + 1)],
                        VT[:, 512 * q + 128 * u : 512 * q + 128 * (u + 1)],
                        start=True,
                        stop=True,
                    )
                cp_engines[(ti + 1) % 2](out_sb[:, ti, :], ops[:])
            nc.sync.dma_start(out=outv[:, :, q * 512 : (q + 1) * 512], in_=out_sb[:])


def build():
    nc = bacc.Bacc()
    pred = nc.dram_tensor("pred", [NS, 5], F32, kind="ExternalInput")
    targ = nc.dram_tensor("targ", [M, 5], F32, kind="ExternalInput")
    out = nc.dram_tensor("out", [NS, M], F16, kind="ExternalOutput")
    with tile.TileContext(nc) as tc:
        _body(tc, pred[:], targ[:], out[:])
    nc.finalize()
    return nc


_NC_CACHE = None


def _get_nc():
    global _NC_CACHE
    if _NC_CACHE is None:
        _NC_CACHE = build()
    return _NC_CACHE


def kernel(kent_pred, kent_target, trace=False, tmpdir=None):
    from concourse.bass_utils import run_bass_kernel_spmd

    nc = _get_nc()
    kent_pred = np.ascontiguousarray(np.asarray(kent_pred, dtype=np.float32))
    kent_target = np.ascontiguousarray(np.asarray(kent_target, dtype=np.float32))
    in_maps = [
        {"pred": kent_pred[i * NS : (i + 1) * NS], "targ": kent_target}
        for i in range(NCORES)
    ]
    res = run_bass_kernel_spmd(
        nc, in_maps, core_ids=list(range(NCORES)), trace=trace, tmpdir=tmpdir
    )
    out = np.concatenate([r["out"] for r in res.results], axis=0).astype(np.float32)
    if trace:
        kernel.last_results = res
    return out


# revision 18
# speedup vs baseline: 1.0661x; 1.0661x over previous
"""Kent-distribution pairwise KLD loss kernel for Trainium2 (8 NeuronCores).

The [N, M] pairwise KLD matrix factors exactly as a rank-11 product
U @ V^T:

  KLD[n, m] = A[n]                                  (pred-row constant)
            + c_b[m]                                (target-row constant)
            - Ex_a[n] . (kappa_b[m] * gamma_b1[m])  (rank 3)
            + <ExxT_a[n], beta_b[m]*(g_b3 g_b3^T - g_b2 g_b2^T)>  (rank 6, sym)

so each core computes 11 features per pred row (its N-shard) and per
target row (replicated), then skinny matmuls [11,128]^T @ [11,512].
N is sharded across the 8 cores (data parallel over predictions).

v2 performance notes (vs the 41us fp32 baseline):
 - features are rounded to fp16 just before the transpose stage, so
   transposes and the main matmuls run at 1 cycle/row instead of fp32's
   LOW/HIGH 4 cycles/row double-pass (error ~5e-4 vs the 2e-2 gate).
 - the PE is warmed with 3 junk matmuls on the identity during the DVE
   feature phase, so transposes + matmuls run at 2.4 GHz instead of the
   HAM-throttled 1.2 GHz.
 - the feature chain is restructured: batched strided-AP muls compute
   all gamma entries in ~9 vector ops (g2 = cp*P + sp*V', g3 = -sp*P +
   cp*V' with P = (-sa, ca ce, ca se), V' = (-se, ce)); |gamma_1|^2 == 1
   and the ln(2pi) cancellation remove further ops; the kappa/lambda
   chain runs on GpSimd in parallel with the gamma chain on Vector.
 - output is written as fp16 (1MB instead of 2MB of HBM writes; the
   host upcasts) - error budget ~1e-3 vs the 2e-2 gate.

Numerics (validated against the jax reference):
 - exp(c_k - c), exp(c_kk - c) are evaluated as exact algebraic ratios:
   l1 = (k^2-k-4b^2)/D * e^-EPS, l2 = 0.5(2k^3-2k^2-2sk-s)/D^2 with
   D = k^2-4b^2, s = 4b^2 (no Exp table; DVE reciprocal instead).
 - exp(c_beta - c) carries e^-kappa <= 4.5e-5 (kappa >= 10) and is
   dropped; with lambda2 == lambda3, ExxT = l2*I + (l1-l2)*g1 g1^T and
   the beta_a*(qa2-qa3) term vanishes.
 - Sin's HW domain is [-pi, pi]: cos(x) = sin(pi/2 - |x|), |x| on DVE.
"""

import sys

import numpy as np

sys.path.insert(0, "/opt/trn_rl_repo")

import concourse.bass as bass  # noqa: E402,F401
import concourse.mybir as mybir  # noqa: E402
import concourse.tile as tile  # noqa: E402
from concourse import bacc  # noqa: E402
from concourse.masks import make_identity  # noqa: E402

F32 = mybir.dt.float32
F32R = mybir.dt.float32r
F16 = mybir.dt.float16
AF = mybir.ActivationFunctionType
ALU = mybir.AluOpType

N = 2048
M = 2048
NCORES = 8
NS = N // NCORES  # 256 pred rows per core
K = 11  # feature rank
GP = NS // 128  # pred row-groups (2)
GT = M // 128  # target row-groups (16)
G = GP + GT  # 18

PI = float(np.pi)
EPS = 1e-6
EM = float(np.exp(-1e-6))  # e^-EPS factor from the reference's den EPS


def _body(tc, pred, targ, out):
    nc = tc.nc
    with (
        tc.tile_pool(name="main", bufs=1) as pool,
        tc.tile_pool(name="junk_psum", bufs=1, space="PSUM") as jpp,
        tc.tile_pool(name="tp_psum", bufs=2, space="PSUM") as tpp,
        tc.tile_pool(name="ut_psum", bufs=1, space="PSUM") as upp,
        tc.tile_pool(name="out_psum", bufs=3, space="PSUM") as opp,
    ):
        def t(shape, tag, dtype=F32):
            return pool.tile([128, *shape], dtype, name=tag, tag=tag)

        def vmul(o, a, b):
            nc.vector.tensor_mul(o, a, b)

        def gmul(o, a, b):
            nc.gpsimd.tensor_mul(o, a, b)

        def gadd(o, a, b):
            nc.gpsimd.tensor_add(o, a, b)

        def gsub(o, a, b):
            nc.gpsimd.tensor_sub(o, a, b)

        def vstt(o, in0, scalar, in1, op0, op1):
            nc.vector.scalar_tensor_tensor(o, in0, scalar, in1, op0, op1)

        # ---- identity (for PE transposes) + PE warm-up.  The HAM
        # clock-gate opens (1.2 -> 2.4 GHz) only after one complete
        # free-running 4096-cycle (3.4us) window of sustained PE
        # activity; 22 junk matmuls (~7us cold) guarantee a full window
        # regardless of phase, so the real transposes/matmuls run warm.
        ident = t([128], "ident", F16)
        make_identity(nc, ident)
        junk = jpp.tile([128, 384], F32, name="junk", tag="junk")
        ident_wide = ident[:].unsqueeze(1).broadcast_to([128, 3, 128])
        for _ in range(16):
            nc.tensor.matmul(junk[:], ident[:], ident_wide, start=True, stop=True)
        # finer-grained tail junks: small over-shoot if the feature phase
        # finishes first, while still bridging the HAM MID window
        for _ in range(28):
            nc.tensor.matmul(
                junk[:, 0:128], ident[:], ident[:], start=True, stop=True
            )

        # ---- load params: pred partition p holds rows 2p,2p+1; targ
        # partition p holds rows 16p..16p+15 (contiguous per-partition DMA).
        params = t([G * 5], "params")
        # j-major input layout: row r lands at (partition r % 128, slot
        # r // 128), so each transposed group j is the contiguous output
        # block [128 j, 128 j + 128) -- VT copies and the main matmul
        # moving operand stay fully contiguous.  The DRAM side becomes a
        # 20 B-granular gather (2048 descriptors ~ 0.9 us across the 16
        # SDMA engines), paid once during the preamble shadow.
        # The two DMAs issue from different engines (Act + SP HWDGE
        # rings) so their ~2us completion latencies overlap.
        nc.scalar.dma_start(
            out=params[:, 0 : GP * 5].rearrange("p (j c) -> p j c", c=5),
            in_=pred.rearrange("(j p) c -> p j c", p=128),
        )
        nc.sync.dma_start(
            out=params[:, GP * 5 : G * 5].rearrange("p (j c) -> p j c", c=5),
            in_=targ.rearrange("(j p) c -> p j c", p=128),
        )

        P5 = params.rearrange("p (g c) -> p c g", c=5)  # [128, 5, 18]
        kap = P5[:, 3, :]  # [128, 18] stride-5 slabs
        bet = P5[:, 4, :]
        kap_p, kap_t = kap[:, 0:GP], kap[:, GP:G]
        bet_t = bet[:, GP:G]

        half_pi = pool.tile([128, 1], F32, name="half_pi", tag="half_pi")
        nc.vector.memset(half_pi, PI / 2)
        eps_c = pool.tile([128, 1], F32, name="eps_c", tag="eps_c")
        nc.vector.memset(eps_c, EPS)
        # dummy Sin on a constant: hoists the trig ACT_TABLE_LOAD off the
        # input-DMA critical path (runs while the DMA is in flight)
        sin_dummy = pool.tile([128, 1], F32, name="sin_dummy", tag="sin_dummy")
        nc.scalar.activation(sin_dummy[:], half_pi[:], AF.Sin)
        # (no dummy Ln: the trig and natural-log act-table sets evict each
        # other, so any Sin->Ln interleave forces 1.28us reloads; the one
        # real Ln below is the single switch this kernel pays for)

        # ---- trig: TRIG rows (se, sa, sp, ce, ca, cp, -se, pad, -sp).
        # The row spacing is chosen so later ops can read pairs like
        # (cp,-sp), (sp,cp), (ce,se), (-se,ce) with stride-3 APs.
        angles = P5[:, 0:3, :]  # [128, 3, 18] (eta, alpha, psi)
        TRIG = t([9, G], "TRIG")
        absv = t([3, G], "absv")
        vstt(absv[:], angles, -1.0, angles, ALU.mult, ALU.max)
        nc.scalar.activation(TRIG[:, 0:3, :], angles, AF.Sin)
        nc.scalar.activation(TRIG[:, 3:6, :], absv[:], AF.Sin, bias=half_pi, scale=-1.0)
        nc.scalar.activation(TRIG[:, 6::2, :], P5[:, 0:3:2, :], AF.Sin, scale=-1.0)
        se = TRIG[:, 0, :]
        sa = TRIG[:, 1, :]
        ce = TRIG[:, 3, :]
        ca = TRIG[:, 4, :]
        ce_se = TRIG[:, 3::-3, :]  # rows (3,0) = (ce, se)
        cp_nsp = TRIG[:, 5::3, :]  # rows (5,8) = (cp, -sp)
        sp_cp = TRIG[:, 2:6:3, :]  # rows (2,5) = (sp, cp)
        nse_ce = TRIG[:, 6:2:-3, :]  # rows (6,3) = (-se, ce)

        # ---- kappa/beta chain on gpsimd (parallel with trig/gamma on
        # vector): c = ln2pi + k - 0.5 ln((k-2b)(k+2b)+EPS); the ln2pi
        # cancels between c_a and c_b and is dropped from both sides.
        b2 = t([G], "b2")
        gadd(b2, bet, bet)  # 2*beta
        b2_t = b2[:, GP:G]
        x2f = t([G], "x2f")
        gmul(x2f, kap, kap)
        b2sq = t([G], "b2sq")
        gmul(b2sq, b2, b2)
        # LNIN = (k-2b)(k+2b) = k^2 - 4b^2; its pred slice IS the lambda
        # denominator D, and x2f/b2sq pred slices are x2/s reused below
        LNIN = t([G], "LNIN")
        gsub(LNIN[:], x2f, b2sq)
        LNOUT = t([G], "LNOUT")
        nc.scalar.activation(LNOUT[:], LNIN[:], AF.Ln, bias=eps_c)
        lnprod_p = LNOUT[:, 0:GP]
        lnprod_t = LNOUT[:, GP:G]

        # pred-only lambda chain prefix (gpsimd).  GpSimd has no
        # scalar_tensor_tensor / tensor_scalar ISA support, so the chain
        # is formulated sign-flipped in pure tensor_tensor ops:
        #   negn = -(k^2 - k - s),  l1n = -l1 = negn/D,
        #   Qhn = k*negn + s/2 = -Q/2,  l2n = -l2 = Qhn/D^2,
        #   dEn = l1n - l2n = -(l1 - l2)
        # (the e^-EPS factor on l1 is dropped: relative 1e-6, far below
        # the fp16 feature rounding)
        b2_p = b2[:, 0:GP]
        bet_p = bet[:, 0:GP]
        x2 = x2f[:, 0:GP]  # k^2
        s4 = b2sq[:, 0:GP]  # s = 4 b^2
        D = LNIN[:, 0:GP]  # k^2 - 4 b^2
        sh = t([GP], "sh")
        gmul(sh, b2_p, bet_p)  # s/2 = 2 b^2
        tneg = t([GP], "tneg")
        gsub(tneg, x2, kap_p)
        negn = t([GP], "negn")
        gsub(negn, s4, tneg)  # -(k^2 - k - s)

        # ---- gamma vectors (vector engine), batched:
        #   g1 = (ca, sa ce, sa se)
        #   g2 = cp*P + sp*V',  g3 = -sp*P + cp*V'
        # with P = (-sa, ca ce, ca se), V' = (-se, ce) (comps 1:3).
        gam = t([3, 3, G], "gam")  # [row, comp, group]
        vmul(gam[:, 0, 1:3, :], sa.unsqueeze(1).broadcast_to([128, 2, G]), ce_se)
        P = t([3, G], "P")
        vmul(P[:, 1:3, :], ca.unsqueeze(1).broadcast_to([128, 2, G]), ce_se)
        nc.vector.tensor_scalar_mul(P[:, 0, :], sa, -1.0)
        nc.vector.tensor_copy(gam[:, 0, 0, :], ca)
        # r = 1/D early (vector owns reciprocal; gpsimd lambda chain waits)
        r = t([GP], "r")
        nc.vector.reciprocal(r, D)
        r2 = t([GP], "r2")
        vmul(r2, r, r)
        # X = (cp*P ; -sp*P) written straight into gam rows 2-3; Y is
        # then added in place on the 1:3 components
        vmul(
            gam[:, 1:3, :, :],
            cp_nsp.unsqueeze(2).broadcast_to([128, 2, 3, G]),
            P[:].unsqueeze(1).broadcast_to([128, 2, 3, G]),
        )
        Y = t([2, 2, G], "Y")  # (sp*V' ; cp*V')
        vmul(
            Y[:],
            sp_cp.unsqueeze(2).broadcast_to([128, 2, 2, G]),
            nse_ce.unsqueeze(1).broadcast_to([128, 2, 2, G]),
        )
        nc.vector.tensor_add(gam[:, 1:3, 1:3, :], gam[:, 1:3, 1:3, :], Y[:])

        # ---- pair products p_c_e for e in [00,11,22,01,02,12]
        prod = t([3, 6, G], "prod")
        vmul(prod[:, :, 0:3, :], gam[:], gam[:])
        vmul(
            prod[:, :, 3:5, :],
            gam[:, :, 0:1, :].broadcast_to([128, 3, 2, G]),
            gam[:, :, 1:3, :],
        )
        vmul(prod[:, :, 5, :], gam[:, :, 1, :], gam[:, :, 2, :])
        # dVn = -(g3 x g3 - g2 x g2); pairs with the negated ExxT rows in UF
        dVn = t([6, GT], "dVn")
        nc.vector.tensor_sub(dVn, prod[:, 1, :, GP:G], prod[:, 2, :, GP:G])

        # ---- gpsimd lambda chain (sign-flipped, see above)
        l1n = t([GP], "l1n")
        gmul(l1n, negn, r[:])  # -l1
        kneg = t([GP], "kneg")
        gmul(kneg, kap_p, negn)
        Qhn = t([GP], "Qhn")
        gadd(Qhn, kneg, sh)  # -0.5 Q
        l2n = t([GP], "l2n")
        gmul(l2n, Qhn, r2[:])  # -l2
        dEn = t([GP], "dEn")
        gsub(dEn, l1n, l2n)  # -(l1 - l2)

        l1nb = l1n[:].unsqueeze(1)
        l2nb = l2n[:].unsqueeze(1)
        dEnb = dEn[:].unsqueeze(1)

        # ---- target features, split into TWO tiles (Tile tracks deps at
        # tile granularity): vector assembles VFa (groups 0-7), gpsimd
        # VFb (groups 8-15), so transposes of the front half need not
        # wait for the back half.  VFb's muls are emitted after the UF
        # section below so UT unblocks first.
        H = GT // 2
        VFa = t([K, H], "VFa", F16)
        VFb = t([K, H], "VFb", F16)
        nc.gpsimd.memset(VFb[:, 0, :], 1.0)
        nc.vector.memset(VFa[:, 0, :], 1.0)
        # c_b (ln2pi dropped; cancels against c_a)
        vstt(VFa[:, 1, :], lnprod_t[:, 0:H], -0.5, kap_t[:, 0:H], ALU.mult, ALU.add)
        vstt(VFb[:, 1, :], lnprod_t[:, H:GT], -0.5, kap_t[:, H:GT], ALU.mult, ALU.add)
        kb3 = kap_t.unsqueeze(1).broadcast_to([128, 3, GT])
        bb3 = bet_t.unsqueeze(1).broadcast_to([128, 3, GT])
        b2b3 = b2_t.unsqueeze(1).broadcast_to([128, 3, GT])
        gam1t = gam[:, 0, :, GP:G]
        vmul(VFa[:, 2:5, :], gam1t[:, :, 0:H], kb3[:, :, 0:H])
        vmul(VFa[:, 5:8, :], dVn[:, 0:3, 0:H], bb3[:, :, 0:H])
        vmul(VFa[:, 8:11, :], dVn[:, 3:6, 0:H], b2b3[:, :, 0:H])

        # ---- pred features UF [128, 11, 2] (gpsimd; a1 on vector, the
        # only op here needing an immediate-scalar ALU)
        UF = t([K, GP], "UF", F16)
        nc.gpsimd.memset(UF[:, 1, :], 1.0)
        # Ex_a with the pairing minus sign folded in: -l1 * g1 = l1n * g1
        vmul(UF[:, 2:5, :], gam[:, 0, :, 0:GP], l1nb.broadcast_to([128, 3, GP]))
        # -ExxT = l2n I + dEn g1 g1^T  (sign matched by dVn on the V side)
        edtn = t([3, GP], "edtn")
        gmul(edtn, prod[:, 0, 0:3, 0:GP], dEnb.broadcast_to([128, 3, GP]))
        gadd(UF[:, 5:8, :], edtn, l2nb.broadcast_to([128, 3, GP]))
        vmul(UF[:, 8:11, :], prod[:, 0, 3:6, 0:GP], dEnb.broadcast_to([128, 3, GP]))
        # A = (0.5 lnprod - k) + k l1  (|gamma_1|^2 == 1 exactly)
        kadotn = t([GP], "kadotn")
        gmul(kadotn, kap_p, l1n)  # -k l1
        a1 = t([GP], "a1")
        vstt(a1, lnprod_p, 0.5, kap_p, ALU.mult, ALU.subtract)
        gsub(UF[:, 0, :], a1, kadotn)

        # back-half target features (gpsimd), after UF so UT unblocks first
        gmul(VFb[:, 2:5, :], gam1t[:, :, H:GT], kb3[:, :, H:GT])
        gmul(VFb[:, 5:8, :], dVn[:, 0:3, H:GT], bb3[:, :, H:GT])
        gmul(VFb[:, 8:11, :], dVn[:, 3:6, H:GT], b2b3[:, :, H:GT])

        # ---- transpose targets to group-major VT [11, 2048] (col = 128j + p)
        # (PSUM -> SBUF copies must stay on scalar/vector: GpSimd has no
        # PSUM access.)  The UT transposes slot between the two VF halves.
        VT = pool.tile([K, M], F16, name="VT", tag="VT")
        UT = pool.tile([K, NS], F16, name="UT", tag="UT")
        cp_engines = [nc.scalar.copy, nc.vector.tensor_copy]

        # with the j-major input layout, group j's transpose IS the
        # contiguous output block m in [128 j, 128 j + 128), and main-
        # matmul chunk c needs only VT quarter c -- so each quarter runs
        # its own transpose -> copy -> matmul -> out-copy -> DMA pipeline
        # instead of waiting for all of VT.

        # preds first: UT col = pred row (j-major load), plain copy
        utp = upp.tile([K, GP * 128], F16, name="utp", tag="utp")
        for j in range(GP):
            nc.tensor.transpose(utp[:, j * 128 : (j + 1) * 128], UF[:, :, j], ident[:])
        nc.vector.tensor_copy(UT[:], utp[:])

        UTr = UT[:]
        outv = out.rearrange("(t p) m -> p t m", p=128)  # row = 128 t + p
        for q in range(4):
            vtp = tpp.tile([K, 512], F16, name="vtp", tag="vtp")
            VFh = VFa if q < 2 else VFb
            for jj in range(4):
                j = (q % 2) * 4 + jj
                nc.tensor.transpose(
                    vtp[:, jj * 128 : (jj + 1) * 128], VFh[:, :, j], ident[:]
                )
            cp_engines[q % 2](VT[:, q * 512 : (q + 1) * 512], vtp[:])
            out_sb = pool.tile([128, GP, 512], F16, name="out_sb", tag="out_sb", bufs=4)
            for ti in range(GP):
                ops = opp.tile([128, 512], F32, name="ops", tag="ops", bufs=4)
                # 4x128-col matmuls pipeline at ~56 ns pace on the PE
                # queue; a single 512-col matmul runs ~374 ns
                for u in range(4):
                    nc.tensor.matmul(
                        ops[:, 128 * u : 128 * (u + 1)],
                        UTr[:, 128 * ti : 128 * (ti + 1)],
                        VT[:, 512 * q + 128 * u : 512 * q + 128 * (u + 1)],
                        start=True,
                        stop=True,
                    )
                cp_engines[(ti + 1) % 2](out_sb[:, ti, :], ops[:])
            nc.sync.dma_start(out=outv[:, :, q * 512 : (q + 1) * 512], in_=out_sb[:])
            if q < 3:
                for _ in range(4):
                    nc.tensor.matmul(
                        junk[:, 0:128], ident[:], ident[:], start=True, stop=True
                    )


def build():
    nc = bacc.Bacc()
    pred = nc.dram_tensor("pred", [NS, 5], F32, kind="ExternalInput")
    targ = nc.dram_tensor("targ", [M, 5], F32, kind="ExternalInput")
    out = nc.dram_tensor("out", [NS, M], F16, kind="ExternalOutput")
    with tile.TileContext(nc) as tc:
        _body(tc, pred[:], targ[:], out[:])
    nc.finalize()
    return nc


_NC_CACHE = None


def _get_nc():
    global _NC_CACHE
    if _NC_CACHE is None:
        _NC_CACHE = build()
    return _NC_CACHE


def kernel(kent_pred, kent_target, trace=False, tmpdir=None):
    from concourse.bass_utils import run_bass_kernel_spmd

    nc = _get_nc()
    kent_pred = np.ascontiguousarray(np.asarray(kent_pred, dtype=np.float32))
    kent_target = np.ascontiguousarray(np.asarray(kent_target, dtype=np.float32))
    in_maps = [
        {"pred": kent_pred[i * NS : (i + 1) * NS], "targ": kent_target}
        for i in range(NCORES)
    ]
    res = run_bass_kernel_spmd(
        nc, in_maps, core_ids=list(range(NCORES)), trace=trace, tmpdir=tmpdir
    )
    out = np.concatenate([r["out"] for r in res.results], axis=0).astype(np.float32)
    if trace:
        kernel.last_results = res
    return out
